# revision 4
# baseline (speedup 1.0000x reference)
"""Bass/Trainium2 kernel for nn_Channel_attention (bottom-16 channel gather).

reference semantics (per sample b):
    weight = mean(x[b], axis=(H, W))           # [C]
    idx    = argsort(weight)[:16]              # ascending pooled value
    out[b] = x[b, idx]                         # [16, H, W]

Strategy: pure data parallel, B=16 sharded 2 samples per core over 8 cores.
Per core (x shard viewed as [512, 16384] = [(sample, channel), H*W]):
  1. Stream 2 MiB load chunks on the SP HWDGE queue into three rotating
     SBUF-resident [128ch, 16384] group tiles, so each sample's data is
     fully in SBUF when its bottom-16 is known.
  2. The Activation engine consumes the stream: activation(Copy,
     accum_out) yields per-channel partial sums at ~1 elem/lane/cycle,
     leaving the Vector engine nearly idle. DVE merges partials
     (negated), PE transposes the 128-channel group sums into a [1, 256]
     row, and two rounds of max8/match_replace select the bottom-16
     pooled values in ascending order (argsort of sum == argsort of mean).
  3. No index gather at all: the 16 selected VALUES are broadcast to all
     partitions with a K=1 matmul, each channel compares its own sum
     against them (is_equal) and derives its output row (rank), with
     non-selected channels mapped out of bounds. One indirect SWDGE
     scatter per (group, column-half) then writes the selected channels
     straight from the resident SBUF tiles to HBM; OOB rows are silently
     skipped. This removes the HBM re-read of gathered channels and the
     separate store of the old gather+store endgame.
"""

import sys

if "/opt/trn_rl_repo" not in sys.path:
    sys.path.insert(0, "/opt/trn_rl_repo")

import numpy as np

from concourse import bacc, mybir, tile
from concourse.bass import IndirectOffsetOnAxis
from concourse.bass_utils import run_bass_kernel_spmd
from concourse.masks import make_identity

N_CORES = 8
B, C, H, W = 16, 256, 128, 128
K = 16
BPC = B // N_CORES          # samples per core = 2
E = H * W                   # 16384 elems per channel
ROWS = BPC * C              # 512 channel rows per core
NG = C // 128               # channel groups (128 partitions) per sample
HALF = E // 2               # scatter column half (32 KiB descriptors)
BIG = 64.0                  # out-of-bounds rank for non-selected channels
                            # (small: huge ranks overflow 32-bit byte
                            # addresses in the SWDGE ucode and wedge the HW)

f32 = mybir.dt.float32
i32 = mybir.dt.int32
X = mybir.AxisListType.X
Alu = mybir.AluOpType
ActFn = mybir.ActivationFunctionType

# 2 MiB load chunks; the final group tapers so the last landing chunk is
# small and the select chain starts as early as possible.
LOAD_CHUNKS = [4096] * 4
LOAD_CHUNKS_LAST = [4096, 4096, 4096, 2048, 1024, 1024]
ACT_W = 2048                # max activation piece (PSUM scratch = 4 banks)

_cache = {}


def _act_pieces(chunks):
    """(offset, width) activation pieces covering the load chunks."""
    pieces = []
    off = 0
    for w in chunks:
        o = 0
        while o < w:
            pw = min(ACT_W, w - o)
            pieces.append((off + o, pw))
            o += pw
        off += w
    return pieces


def _build():
    nc = bacc.Bacc("TRN2", target_bir_lowering=False, debug=False,
                   num_devices=N_CORES)
    x_d = nc.dram_tensor("x", [ROWS, E], f32, kind="ExternalInput")
    # y rows are (sample, rank, column-half): [2*16*2, 8192]. Declared
    # half-width so the indirect scatter's coef/pitch is a legal <64 KiB
    # descriptor and the indirect-side AP offset is 0.
    y_d = nc.dram_tensor("y", [BPC * K * 2, HALF], f32,
                         kind="ExternalOutput")

    with tile.TileContext(nc) as tc:
        with (
            tc.tile_pool(name="data", bufs=3) as data_pool,
            tc.tile_pool(name="small", bufs=1) as small,
            tc.tile_pool(name="psum", bufs=1, space="PSUM") as psum,
        ):
            # ---- constants (no deps; fill scheduler gaps at startup) ----
            ident = small.tile([128, 128], f32)
            make_identity(nc, ident[:])

            ones_i = small.tile([1, 128], i32)
            nc.gpsimd.iota(out=ones_i[:], pattern=[[1, 128]], base=0,
                           channel_multiplier=0)
            ones_row = small.tile([1, 128], f32)
            nc.vector.tensor_scalar(out=ones_row[:], in0=ones_i[:],
                                    scalar1=0.0, scalar2=1.0,
                                    op0=Alu.mult, op1=Alu.add)

            jrow_i = small.tile([128, K], i32)
            nc.gpsimd.iota(out=jrow_i[:], pattern=[[1, K]], base=0,
                           channel_multiplier=0)
            jrow = small.tile([128, K], f32)
            nc.vector.tensor_copy(jrow[:], jrow_i[:])

            # shared select scratch (DVE-serial; safe to reuse across samples)
            w_neg = small.tile([1, C], f32, tag="wneg")
            w_rep = small.tile([1, C], f32, tag="wrep")
            m1 = small.tile([1, 8], f32, tag="m1")
            m2 = small.tile([1, 8], f32, tag="m2")
            m_all = small.tile([1, K], f32, tag="mall")
            psum_w = psum.tile([1, C], f32, tag="psw")
            psum_b = psum.tile([128, K], f32, tag="psb")

            for s in range(BPC):
                v = []          # negated group sums, per group
                tiles = []      # resident data tiles, per group
                for g in range(NG):
                    last = (s == BPC - 1 and g == NG - 1)
                    chunks = LOAD_CHUNKS_LAST if last else LOAD_CHUNKS
                    pieces = _act_pieces(chunks)
                    base = s * C + g * 128

                    t = data_pool.tile([128, E], f32)
                    tiles.append(t)
                    partials = small.tile([128, len(pieces)], f32,
                                          tag=f"part{s}_{g}")

                    # interleave emission so each activation piece only
                    # depends on loads at or before its own chunk
                    pi = 0
                    off = 0
                    for w in chunks:
                        nc.sync.dma_start(out=t[:, off:off + w],
                                          in_=x_d[base:base + 128,
                                                  off:off + w])
                        off += w
                        while pi < len(pieces) and \
                                pieces[pi][0] + pieces[pi][1] <= off:
                            po, pw = pieces[pi]
                            scr = psum.tile([128, ACT_W], f32, tag="actscr")
                            nc.scalar.activation(
                                out=scr[:, 0:pw], in_=t[:, po:po + pw],
                                func=ActFn.Copy,
                                accum_out=partials[:, pi:pi + 1])
                            pi += 1

                    vg = small.tile([128, 1], f32, tag=f"v{s}_{g}")
                    nc.vector.reduce_sum(out=vg[:], in_=partials[:],
                                         axis=X, negate=True)
                    v.append(vg)
                    # transpose group sums into row layout for max8
                    nc.tensor.matmul(out=psum_w[:, g * 128:(g + 1) * 128],
                                     lhsT=vg[:], rhs=ident[:],
                                     start=True, stop=True)

                # ---- bottom-16 select on negated sums (values only) ----
                nc.vector.tensor_copy(w_neg[:], psum_w[:])
                nc.vector.max(out=m1[:], in_=w_neg[:])
                nc.vector.match_replace(out=w_rep[:], in_to_replace=m1[:],
                                        in_values=w_neg[:], imm_value=-1e38)
                nc.vector.max(out=m2[:], in_=w_rep[:])
                nc.vector.tensor_copy(m_all[:, 0:8], m1[:])
                nc.vector.tensor_copy(m_all[:, 8:16], m2[:])
                # broadcast the 16 selected values to all 128 partitions
                nc.tensor.matmul(out=psum_b[:], lhsT=ones_row[:],
                                 rhs=m_all[:], start=True, stop=True)

                # ---- per group: rank each channel, scatter from SBUF ----
                for g in range(NG):
                    eq = small.tile([128, K], f32, tag=f"eq{g}")
                    nc.vector.tensor_scalar(out=eq[:], in0=psum_b[:],
                                            scalar1=v[g][:], scalar2=None,
                                            op0=Alu.is_equal)
                    tj = small.tile([128, K], f32, tag=f"tj{g}")
                    nc.vector.tensor_tensor(out=tj[:], in0=eq[:],
                                            in1=jrow[:], op=Alu.mult)
                    matched = small.tile([128, 1], f32, tag=f"match{g}")
                    nc.vector.reduce_sum(out=matched[:], in_=eq[:], axis=X)
                    r1 = small.tile([128, 1], f32, tag=f"r1{g}")
                    nc.vector.reduce_sum(out=r1[:], in_=tj[:], axis=X)
                    # rank = r1 + s*K if matched else >= BIG
                    penal = small.tile([128, 1], f32, tag=f"pen{g}")
                    nc.vector.tensor_scalar(out=penal[:], in0=matched[:],
                                            scalar1=float(s * K) - BIG,
                                            scalar2=BIG,
                                            op0=Alu.mult, op1=Alu.add)
                    rank = small.tile([128, 1], f32, tag=f"rank{g}")
                    nc.vector.tensor_tensor(out=rank[:], in0=r1[:],
                                            in1=penal[:], op=Alu.add)
                    for h in range(2):
                        offs = small.tile([128, 1], i32,
                                          tag=f"offs{s}_{g}_{h}")
                        nc.vector.tensor_scalar(out=offs[:], in0=rank[:],
                                                scalar1=2.0,
                                                scalar2=float(h),
                                                op0=Alu.mult, op1=Alu.add)
                        nc.gpsimd.indirect_dma_start(
                            out=y_d[:],
                            out_offset=IndirectOffsetOnAxis(ap=offs[:],
                                                            axis=0),
                            in_=tiles[g][:, h * HALF:(h + 1) * HALF],
                            in_offset=None,
                            bounds_check=BPC * K * 2 - 1,
                            oob_is_err=False)

    nc.compile()
    return nc


def get_nc():
    if "nc" not in _cache:
        _cache["nc"] = _build()
    return _cache["nc"]


def make_in_maps(x: np.ndarray) -> list[dict[str, np.ndarray]]:
    x = np.ascontiguousarray(np.asarray(x, dtype=np.float32))
    assert x.shape == (B, C, H, W)
    return [{"x": x[c * BPC:(c + 1) * BPC].reshape(ROWS, E)}
            for c in range(N_CORES)]


def assemble(results: list[dict[str, np.ndarray]]) -> np.ndarray:
    out = np.empty((B, K, H, W), dtype=np.float32)
    for c in range(N_CORES):
        out[c * BPC:(c + 1) * BPC] = results[c]["y"].reshape(BPC, K, H, W)
    return out


def kernel(x: np.ndarray) -> np.ndarray:
    nc = get_nc()
    res = run_bass_kernel_spmd(nc, make_in_maps(x), list(range(N_CORES)))
    return assemble(res.results)
